# revision 12
# baseline (speedup 1.0000x reference)
"""CenterLoss kernel for Trainium2 (8 NeuronCores, SPMD data-parallel).

Reference computes
    distmat[b,c] = ||x_b||^2 + ||c_c||^2 - 2<x_b, c_c>          [B, C]
    loss = sum(clip(distmat * onehot(labels), 1e-12, 1e12)) / B

Only distmat[b, labels[b]] survives the mask; each of the B*(C-1) masked
zeros becomes exactly 1e-12 under the clip. So instead of the [8192, 10000]
distmat (42 GFLOP), each core gathers its rows' centers and computes per-row
squared distances; the host adds the closed-form constant B*(C-1)*1e-12 and
divides by B.

Sharding: batch split 8 ways (1024 rows/core), centers replicated, all data
bf16 (rounding nets out to ~1e-5 on the loss; tolerance is 2e-2).

Per-core kernel:
  - labels as an int16 [128, 64] tile in dma_gather's wrapped layout
    (idx16[16k+q, s] = label[s*16+q], replicated across the 8 ucode cores)
  - TWO InstDMAGatherAnt ops (512 rows each) fetch all 1024 center rows:
    ct[p, g, :] = centers[label[g*128+p], :].  One SWDGE instruction costs
    ~1us fixed + ~0.34ns/descriptor, so 2 x 512 descriptors beats the
    8 x 128 indirect_dma_start chain (~11.3us) by ~8us.  Two instead of one
    so the second half's transfer overlaps the first half's compute.
  - x loaded bf16 in the matching (g p) row order, halves on the Scalar
    engine's HWDGE queue behind the labels load (same queue => the 4KB
    labels transfer is not stuck behind 1MB of x on the shared DMA queues).
  - per half: one wide DVE subtract [128, 4*256] and one DVE
    scalar_tensor_tensor (dt*dt with accum_out) giving the half-sum per
    partition.  No ACT engine => no act-table load, fewer const memsets.
  - [128, 2] f32 partial sums DMA'd out; host sums and adds the constant.

Hard-won HW constraints baked in here (this runtime rejects/crashes
otherwise):
  - Use Bacc, and finalize() before run: TRN2 codegen allows ONE sync-wait
    per instruction; Bacc's generate_event_semaphores splits multi-waits,
    and the bass2jax path serializes the module without finalizing.
  - indirect_dma_start generates ONE descriptor per offset-AP partition
    (128/instruction; extra offset columns are silently ignored, each
    descriptor copying dest_free_size contiguous elements).  dma_gather
    (InstDMAGatherAnt) DOES work on this runtime — the previous note here
    claiming it kills the exec unit was wrong (verified by probe; the DVE
    READ_ACCUMULATOR2_ANT opcode also runs fine).
  - No in-place DVE ops (out aliasing an input) — exec-unit crash.
  - Bass.__init__ emits 4 const-tile memsets that would otherwise be the
    first "useful" instructions in the NEFF's measured span; they are dead
    code here and are stripped (see _strip_dead_const_memsets).
"""

import ml_dtypes
import numpy as np

from concourse import bacc, bass, mybir
import concourse.tile as tile
from concourse.bass_utils import run_bass_kernel_spmd

B = 8192
C = 10000
D = 256
N_CORES = 8
BL = B // N_CORES  # rows per core
P = 128            # SBUF partitions
G = BL // P        # row groups per core
H = G // 2         # groups per gather half

_CLIP_LO = 1e-12

_nc_cache = None


def _strip_dead_const_memsets(nc):
    """Bass.__init__ unconditionally memsets 4 const tiles (f32 0/1, bf16 1,
    u8 127). This kernel only uses immediate scalars, so they are dead code —
    and they sit before the first real instruction, so dropping them also
    drops them from the NEFF's measured span. Assert nothing references them
    before removing."""
    for func in nc.m.functions:
        for bb in func.blocks:
            for inst in bb.instructions:
                if type(inst).__name__ == "InstMemset":
                    continue
                for ap in list(inst.ins or []) + list(inst.outs or []):
                    memref = getattr(ap, "memref", "") or ""
                    assert not memref.startswith("const-"), (inst.name, memref)
    bb = nc.main_func.blocks[0]
    bb.instructions[:] = [
        inst
        for inst in bb.instructions
        if not (
            type(inst).__name__ == "InstMemset"
            and (inst.outs[0].memref or "").startswith("const-")
        )
    ]


def _build():
    global _nc_cache
    if _nc_cache is not None:
        return _nc_cache

    nc = bacc.Bacc()
    x_l = nc.dram_tensor("x_local", [BL, D], mybir.dt.bfloat16, kind="ExternalInput")
    lab16 = nc.dram_tensor(
        "lab16", [P, BL // 16], mybir.dt.int16, kind="ExternalInput"
    )
    cen = nc.dram_tensor("centers", [C, D], mybir.dt.bfloat16, kind="ExternalInput")
    out = nc.dram_tensor("partials", [P, 2], mybir.dt.float32, kind="ExternalOutput")

    with tile.TileContext(nc) as tc:
        with (
            tc.tile_pool(name="big", bufs=1) as big,
            tc.tile_pool(name="work", bufs=2) as work,
        ):
            lt = big.tile([P, BL // 16], mybir.dt.int16)
            xt = big.tile([P, G, D], mybir.dt.bfloat16)
            ct = big.tile([P, G, D], mybir.dt.bfloat16)
            acc = big.tile([P, 2], mybir.dt.float32)

            # labels first on the Scalar HWDGE queue (Sync is stuck ~700ns
            # in its entry drain); x halves behind it on the same queue.
            nc.scalar.dma_start(out=lt[:], in_=lab16[:])
            x_ap = x_l[:].rearrange("(g p) d -> p g d", p=P)
            nc.scalar.dma_start(out=xt[:, 0:H, :], in_=x_ap[:, 0:H, :])
            nc.scalar.dma_start(out=xt[:, H:, :], in_=x_ap[:, H:, :])

            for h in range(2):
                gsl = slice(h * H, (h + 1) * H)
                nc.gpsimd.dma_gather(
                    out_ap=ct[:, gsl, :],
                    in_ap=cen[:],
                    idxs_ap=lt[:, h * (BL // 32):(h + 1) * (BL // 32)],
                    num_idxs=BL // 2,
                    num_idxs_reg=BL // 2,
                    elem_size=D,
                )
                dt = work.tile([P, H * D], mybir.dt.bfloat16, tag="dt")
                nc.vector.tensor_sub(
                    out=dt[:],
                    in0=xt[:, gsl, :].rearrange("p g d -> p (g d)"),
                    in1=ct[:, gsl, :].rearrange("p g d -> p (g d)"),
                )
                sq = work.tile([P, H * D], mybir.dt.bfloat16, tag="sq")
                nc.vector.scalar_tensor_tensor(
                    out=sq[:],
                    in0=dt[:],
                    scalar=1.0,
                    in1=dt[:],
                    op0=mybir.AluOpType.mult,
                    op1=mybir.AluOpType.mult,
                    accum_out=acc[:, h:h + 1],
                )
            nc.scalar.dma_start(out=out[:], in_=acc[:])

    _strip_dead_const_memsets(nc)

    nc.finalize()
    _nc_cache = nc
    return nc


def _wrap_idx16(idx):
    """dma_gather index layout: [128, n//16] int16, idx16[16k+q, s] =
    idx[s*16+q], replicated across the 8 sixteen-partition ucode cores."""
    n = idx.shape[0]
    w = idx.reshape(n // 16, 16).T.astype(np.int16)
    return np.ascontiguousarray(np.tile(w, (8, 1)))


def _run(x, labels, centers, **spmd_kwargs):
    nc = _build()
    # bf16 inputs halve DMA traffic; |rounding| ~0.4% per element averages
    # out over 8192 rows (net ~1e-5 on the loss, tolerance is 2e-2)
    x = np.ascontiguousarray(np.asarray(x), dtype=np.float32).astype(ml_dtypes.bfloat16)
    labels = np.ascontiguousarray(np.asarray(labels)).astype(np.int32)
    centers = np.ascontiguousarray(np.asarray(centers), dtype=np.float32).astype(
        ml_dtypes.bfloat16
    )

    in_maps = []
    for c in range(N_CORES):
        sl = slice(c * BL, (c + 1) * BL)
        in_maps.append(
            {
                "x_local": x[sl],
                "lab16": _wrap_idx16(labels[sl]),
                "centers": centers,
            }
        )
    res = run_bass_kernel_spmd(nc, in_maps, list(range(N_CORES)), **spmd_kwargs)
    partials = np.stack([r["partials"] for r in res.results])  # [8, P, 2]
    # per-row clip(., 1e-12) is a no-op for this data (distances ~512); the
    # masked zeros' clip contribution is the closed-form constant below
    loss = (partials.astype(np.float64).sum() + B * (C - 1) * _CLIP_LO) / B
    return np.asarray(loss, dtype=np.float32), res


def kernel(x, labels, centers):
    loss, _ = _run(x, labels, centers)
    return loss
